# revision 36
# baseline (speedup 1.0000x reference)
"""GNN message-passing kernel for Trainium2 (8 NeuronCores).

reference:
    m      = relu(concat(x_i, x_j) @ W + b)          # [E, d]
    aggr_m = segment_sum(m, recipients, N_NODES)     # [N, d]
    returns (aggr_m, m)

Strategy:
  * Host: stable-sort edges by recipient; shard by node range (each core owns
    a contiguous block of N/8 nodes => disjoint outputs, no collective).
  * Host: pack each core's sorted edges into 128-edge tiles such that every
    tile's recipients fall inside one 128-node window; the tile->window
    structure is made identical across cores (K_g = max over cores) so a
    single SPMD program serves all 8 cores; padding slots are neutralized
    purely by data (one-hot offset = -1).
  * Device per tile: MLP matmul (lhsT = xcat^T tile [128k, 128e], rhs = W
    [128k, 64]), bias add (DVE) + relu (ACT), one-hot build
    (is_equal(iota, offs)) and scatter matmul (lhsT = onehot [128e, 128w],
    rhs = m [128e, 64]) accumulated in PSUM per node window, flushed to an
    SBUF staging buffer and DMA'd out once.
  * Host: un-permute m, concatenate per-core aggr blocks.
"""

import math
import sys
from contextlib import ExitStack

import numpy as np

for _p in ("/opt/trn_rl_repo",):
    if _p not in sys.path:
        sys.path.insert(0, _p)

import concourse.bass as bass  # noqa: E402
import concourse.mybir as mybir  # noqa: E402
import concourse.tile as tile  # noqa: E402
from concourse import bacc  # noqa: E402
from concourse.bass_utils import run_bass_kernel_spmd  # noqa: E402

P = 128            # partitions / edges per tile
D = 64             # feature dim
WIN = 128          # nodes per scatter window
MGROUP = 8         # tiles per MLP psum group ([128, 512] = one psum bank)
CHUNK = 64         # tiles per input DMA chunk (2 MiB bf16)

F32 = mybir.dt.float32
BF16 = mybir.dt.bfloat16

# compute dtype for x/W/onehot/m ("f32" or "bf16")
X_DTYPE = BF16
M_DTYPE = BF16


def _np_of(dt):
    return np.dtype(mybir.dt.np(dt))


# ---------------------------------------------------------------------------
# host-side packing
# ---------------------------------------------------------------------------

def _pack(x_i, x_j, recipients, n_nodes, n_cores):
    """Sort/shard/pack edges. Returns (per_core list of dicts, meta dict)."""
    E, d = x_i.shape
    assert d == D
    nodes_per_core = n_nodes // n_cores
    assert nodes_per_core * n_cores == n_nodes
    n_win = math.ceil(nodes_per_core / WIN)

    r = np.asarray(recipients).astype(np.int64).ravel()
    order = np.argsort(r, kind="stable").astype(np.int64)
    r_sorted = r[order]
    core_bounds = np.searchsorted(
        r_sorted, np.arange(n_cores + 1) * nodes_per_core
    )

    # window edge counts per (core, window)
    counts = np.zeros((n_cores, n_win), dtype=np.int64)
    per_core_raw = []
    for c in range(n_cores):
        lo, hi = core_bounds[c], core_bounds[c + 1]
        seg_edges = order[lo:hi]
        ln = r_sorted[lo:hi] - c * nodes_per_core      # local node ids
        win = ln // WIN
        offs = ln - win * WIN
        counts[c] = np.bincount(win, minlength=n_win)
        per_core_raw.append((seg_edges, win, offs))

    # Each core orders its windows by descending edge count; program position
    # g holds every core's g-th largest window, so K_g = max over cores of
    # similarly-ranked counts stays tight (less padding than natural order).
    win_perm = np.argsort(-counts, axis=1, kind="stable")  # [c, pos] -> window
    counts_sorted = -np.sort(-counts, axis=1)
    k_g = np.maximum(np.ceil(counts_sorted / P).astype(np.int64).max(axis=0), 1)
    T = int(k_g.sum())
    T_pad = math.ceil(T / CHUNK) * CHUNK
    n_chunks = T_pad // CHUNK
    n_groups = T_pad // MGROUP

    # tile -> window map (pad tiles attach to the last window)
    tile_window = np.repeat(np.arange(n_win), k_g)
    tile_window = np.concatenate(
        [tile_window, np.full(T_pad - T, n_win - 1, dtype=np.int64)]
    )
    # first/last tile per window (over the padded tile list)
    tile_first = np.zeros(T_pad, dtype=bool)
    tile_last = np.zeros(T_pad, dtype=bool)
    for g in range(n_win):
        idx = np.nonzero(tile_window == g)[0]
        tile_first[idx[0]] = True
        tile_last[idx[-1]] = True

    win_slot0 = np.concatenate([[0], np.cumsum(k_g)]) * P  # slot base per window

    x_np = _np_of(X_DTYPE)
    per_core = []
    for c in range(n_cores):
        seg_edges, win, offs = per_core_raw[c]
        cnt = counts[c]
        rank = np.empty(n_win, dtype=np.int64)  # window -> program position
        rank[win_perm[c]] = np.arange(n_win)
        win_starts = np.concatenate([[0], np.cumsum(cnt)])[:-1]
        pos_in_win = np.arange(len(seg_edges)) - np.repeat(win_starts, cnt)
        slot = win_slot0[rank[win]] + pos_in_win

        slot_edge = np.full(T_pad * P, -1, dtype=np.int64)
        slot_off = np.full(T_pad * P, -1.0, dtype=np.float32)
        slot_edge[slot] = seg_edges
        slot_off[slot] = offs.astype(np.float32)

        xs = np.zeros((T_pad * P, 2 * D), dtype=np.float32)
        valid = slot_edge >= 0
        ve = slot_edge[valid]
        xs[valid, :D] = x_i[ve]
        xs[valid, D:] = x_j[ve]
        # chunk-transposed layout: [n_chunks, 128(k), CHUNK*128(e)]
        xcat = (
            xs.reshape(n_chunks, CHUNK, P, 2 * D)
            .transpose(0, 3, 1, 2)
            .reshape(n_chunks, 2 * D, CHUNK * P)
            .astype(x_np)
        )
        offsT = np.ascontiguousarray(slot_off.reshape(T_pad, P).T).astype(
            x_np
        )  # [128, T_pad]
        per_core.append(
            dict(xcat=xcat, offsT=offsT, slot_edge=slot_edge, win_perm=win_perm[c])
        )

    meta = dict(
        T_pad=T_pad,
        n_chunks=n_chunks,
        n_groups=n_groups,
        n_win=n_win,
        nodes_per_core=nodes_per_core,
        tile_window=tile_window,
        tile_first=tile_first,
        tile_last=tile_last,
    )
    return per_core, meta


# ---------------------------------------------------------------------------
# device program
# ---------------------------------------------------------------------------

def _build_program(meta):
    T_pad = meta["T_pad"]
    n_chunks = meta["n_chunks"]
    n_groups = meta["n_groups"]
    n_win = meta["n_win"]
    tile_window = meta["tile_window"]
    tile_first = meta["tile_first"]
    tile_last = meta["tile_last"]

    nc = bacc.Bacc(None)
    xcat_h = nc.declare_dram_parameter(
        "xcat", [n_chunks, 2 * D, CHUNK * P], X_DTYPE, isOutput=False
    )
    offs_h = nc.declare_dram_parameter("offs", [P, T_pad], X_DTYPE, isOutput=False)
    w_h = nc.declare_dram_parameter("w", [2 * D, D], X_DTYPE, isOutput=False)
    bias_h = nc.declare_dram_parameter(
        "bias", [1, MGROUP * D], X_DTYPE, isOutput=False
    )
    iota_h = nc.declare_dram_parameter(
        "iota", [P, MGROUP * WIN], X_DTYPE, isOutput=False
    )
    assert n_groups % 2 == 0
    m_out_h = nc.declare_dram_parameter(
        "m_out", [n_groups // 2, P, 2 * MGROUP * D], M_DTYPE, isOutput=True
    )
    aggr_h = nc.declare_dram_parameter("aggr", [WIN, n_win * D], F32, isOutput=True)

    with tile.TileContext(nc) as tc, ExitStack() as ctx:
        const_pool = ctx.enter_context(tc.tile_pool(name="const", bufs=1))
        chunk_pool = ctx.enter_context(tc.tile_pool(name="xchunk", bufs=4))
        m_pool = ctx.enter_context(tc.tile_pool(name="m", bufs=4))
        oh_pool = ctx.enter_context(tc.tile_pool(name="onehot", bufs=6))
        mpsum_pool = ctx.enter_context(
            tc.tile_pool(name="mpsum", bufs=4, space="PSUM")
        )
        apsum_pool = ctx.enter_context(
            tc.tile_pool(name="apsum", bufs=3, space="PSUM")
        )

        w_sb = const_pool.tile([2 * D, D], X_DTYPE)
        nc.sync.dma_start(out=w_sb[:], in_=w_h[:, :])
        bias_sb = const_pool.tile([1, MGROUP * D], X_DTYPE)
        nc.sync.dma_start(out=bias_sb[:], in_=bias_h[:, :])
        iota_sb = const_pool.tile([P, MGROUP * WIN], X_DTYPE)
        nc.sync.dma_start(out=iota_sb[:], in_=iota_h[:, :])
        offs_sb = const_pool.tile([P, T_pad], X_DTYPE)
        nc.sync.dma_start(out=offs_sb[:], in_=offs_h[:, :])
        stage_sb = const_pool.tile([WIN, n_win * D], F32)
        ones_sb = const_pool.tile([1, P], X_DTYPE)
        nc.vector.memset(ones_sb[:], 1.0)

        chunk_tiles: dict[int, object] = {}
        m_tiles: dict[int, object] = {}
        oh_tiles: dict[int, object] = {}
        aggr_psum = [None]

        def xslice(t):
            ch = t // CHUNK
            if ch not in chunk_tiles:
                xt = chunk_pool.tile([2 * D, CHUNK * P], X_DTYPE)
                nc.sync.dma_start(out=xt[:], in_=xcat_h[ch])
                chunk_tiles[ch] = xt
            j = t % CHUNK
            return chunk_tiles[ch][:, j * P : (j + 1) * P]

        def emit_mlp(g):
            pm = mpsum_pool.tile([P, MGROUP * D], F32)
            # bias pre-load: ones[1,128].T @ bias[1,512] broadcasts b into psum
            nc.tensor.matmul(
                out=pm[:], lhsT=ones_sb[:], rhs=bias_sb[:], start=True, stop=False
            )
            for j in range(MGROUP):
                t = g * MGROUP + j
                nc.tensor.matmul(
                    out=pm[:, j * D : (j + 1) * D],
                    lhsT=xslice(t),
                    rhs=w_sb[:],
                    start=False,
                    stop=(j == MGROUP - 1),
                )
            # m tiles are paired [128, 1024]: group g occupies half (g%2), one
            # DMA per pair -> 2 KiB descriptors instead of 1 KiB
            if g % 2 == 0:
                mpair = m_pool.tile([P, 2 * MGROUP * D], M_DTYPE)
                m_tiles[g // 2] = mpair
            else:
                mpair = m_tiles[g // 2]
            off = (g % 2) * MGROUP * D
            nc.scalar.activation(
                out=mpair[:, off : off + MGROUP * D],
                in_=pm[:],
                func=mybir.ActivationFunctionType.Relu,
            )
            if g % 2 == 1:
                nc.scalar.dma_start(out=m_out_h[g // 2], in_=mpair[:])
            # one-hot build; two halves so the first scatter matmuls can
            # start after half the compare
            oh = oh_pool.tile([P, MGROUP * WIN], X_DTYPE)
            half = MGROUP // 2
            for h in range(2):
                t0 = g * MGROUP + h * half
                nc.vector.tensor_tensor(
                    out=oh[:, h * half * WIN : (h + 1) * half * WIN].rearrange(
                        "p (t w) -> p t w", w=WIN
                    ),
                    in0=iota_sb[:, : half * WIN].rearrange("p (t w) -> p t w", w=WIN),
                    in1=offs_sb[:, t0 : t0 + half].to_broadcast([P, half, WIN]),
                    op=mybir.AluOpType.is_equal,
                )
            oh_tiles[g] = oh

        def emit_scatter(g):
            mpair = m_tiles[g // 2]
            moff = (g % 2) * MGROUP * D
            if g % 2 == 1:
                del m_tiles[g // 2]
            oh = oh_tiles.pop(g)
            for j in range(MGROUP):
                t = g * MGROUP + j
                gw = int(tile_window[t])
                if tile_first[t]:
                    aggr_psum[0] = apsum_pool.tile([WIN, D], F32, name="aggr_psum")
                nc.tensor.matmul(
                    out=aggr_psum[0][:],
                    lhsT=oh[:, j * WIN : (j + 1) * WIN],
                    rhs=mpair[:, moff + j * D : moff + (j + 1) * D],
                    start=bool(tile_first[t]),
                    stop=bool(tile_last[t]),
                )
                if tile_last[t]:
                    nc.scalar.copy(
                        out=stage_sb[:, gw * D : (gw + 1) * D], in_=aggr_psum[0][:]
                    )

        LAG = 3
        for g in range(n_groups):
            emit_mlp(g)
            if g >= LAG:
                emit_scatter(g - LAG)
        for g in range(n_groups - LAG, n_groups):
            emit_scatter(g)

        nc.scalar.dma_start(out=aggr_h[:, :], in_=stage_sb[:])

    nc.compile()
    return nc


# ---------------------------------------------------------------------------
# entry point
# ---------------------------------------------------------------------------

def _make_in_maps(per_core, W, b):
    x_np = _np_of(X_DTYPE)
    w_in = np.ascontiguousarray(W.astype(x_np))
    bias_in = np.tile(b[None, :], (1, MGROUP)).astype(x_np)
    iota_in = np.tile(np.arange(WIN, dtype=np.float64), (P, MGROUP)).astype(x_np)
    return [
        {
            "xcat": pc["xcat"],
            "offs": pc["offsT"],
            "w": w_in,
            "bias": bias_in,
            "iota": iota_in,
        }
        for pc in per_core
    ]


def _decode(results, per_core, meta, E, n_nodes):
    n_win = meta["n_win"]
    npc = meta["nodes_per_core"]
    n_groups = meta["n_groups"]
    aggr = np.zeros((n_nodes, D), dtype=np.float32)
    m = np.empty((E, D), dtype=np.float32)
    for c, out in enumerate(results):
        aggr_np = np.asarray(out["aggr"], dtype=np.float32)  # [WIN, n_win*64]
        blocks = aggr_np.reshape(WIN, n_win, D).transpose(1, 0, 2)  # [pos, WIN, 64]
        win_perm = per_core[c]["win_perm"]
        for g in range(n_win):
            w = int(win_perm[g])
            nv = min(WIN, npc - w * WIN)
            aggr[c * npc + w * WIN : c * npc + w * WIN + nv] = blocks[g, :nv]
        m_np = np.asarray(out["m_out"], dtype=np.float32)  # [n_groups/2,128,1024]
        rows = (
            m_np.reshape(n_groups // 2, P, 2 * MGROUP, D)
            .transpose(0, 2, 1, 3)
            .reshape(n_groups * MGROUP * P, D)
        )
        slot_edge = per_core[c]["slot_edge"]
        valid = slot_edge >= 0
        m[slot_edge[valid]] = rows[valid]
    return aggr, m


def _ensure_ntff_hook():
    """Register the axon NTFF profiling hook under ``antenv.axon_hooks``.

    The agent image's ``antenv`` lacks ``axon_hooks`` so trn_boot degrades
    silently; replicate the registration here so trace=True yields
    exec_time_ns. Best-effort — failure just means no trace."""
    import types

    try:
        import antenv.axon_hooks  # noqa: F401
        return
    except ImportError:
        pass
    try:
        import antenv
        from trn_agent_boot.trn_boot import _ntff_profile_via_ctypes

        hook = _ntff_profile_via_ctypes("/opt/axon/libaxon_pjrt.so")
        mod = types.ModuleType("antenv.axon_hooks")
        mod._hook = hook
        mod.get_axon_ntff_profile_hook = lambda: mod._hook
        mod.set_axon_ntff_profile_hook = lambda h: setattr(mod, "_hook", h)
        sys.modules["antenv.axon_hooks"] = mod
        antenv.axon_hooks = mod
    except Exception as e:  # pragma: no cover
        print(f"NTFF hook shim failed ({e}); tracing disabled", file=sys.stderr)


def _run(x_i, x_j, recipients, W, b, n_nodes, n_cores, trace=False):
    if trace:
        _ensure_ntff_hook()
    x_i = np.asarray(x_i, dtype=np.float32)
    x_j = np.asarray(x_j, dtype=np.float32)
    W = np.asarray(W, dtype=np.float32)
    b = np.asarray(b, dtype=np.float32)
    E = x_i.shape[0]

    per_core, meta = _pack(x_i, x_j, recipients, n_nodes, n_cores)
    nc = _build_program(meta)
    in_maps = _make_in_maps(per_core, W, b)

    res = run_bass_kernel_spmd(nc, in_maps, list(range(n_cores)), trace=trace)
    aggr, m = _decode(res.results, per_core, meta, E, n_nodes)
    return (aggr, m), res


def kernel(x_i, x_j, recipients, W, b):
    (aggr, m), _ = _run(x_i, x_j, recipients, W, b, n_nodes=50000, n_cores=8)
    return aggr, m


# revision 37
# speedup vs baseline: 1.1044x; 1.1044x over previous
"""GNN message-passing kernel for Trainium2 (8 NeuronCores).

reference:
    m      = relu(concat(x_i, x_j) @ W + b)          # [E, d]
    aggr_m = segment_sum(m, recipients, N_NODES)     # [N, d]
    returns (aggr_m, m)

Strategy:
  * Host: stable-sort edges by recipient; shard by node range (each core owns
    a contiguous block of N/8 nodes => disjoint outputs, no collective).
  * Host: pack each core's sorted edges into 128-edge tiles such that every
    tile's recipients fall inside one 128-node window; the tile->window
    structure is made identical across cores (K_g = max over cores) so a
    single SPMD program serves all 8 cores; padding slots are neutralized
    purely by data (one-hot offset = -1).
  * Device per 8-tile group: bias preloaded into PSUM via a K=1 matmul
    (ones[1,128] x bias[1,512]), 8 MLP matmuls (lhsT = xcat^T tile
    [128k, 128e], rhs = W) accumulate on top, ACT relu(PSUM)->SBUF bf16,
    DVE one-hot build (is_equal(iota, offs) with broadcast), scatter
    matmuls (lhsT = onehot [128e, 128w], rhs = m [128e, 64]) accumulated
    in a PSUM window, flushed to staging and DMA'd out once. All compute
    in bf16 with fp32 PSUM accumulation.
  * Host: un-permute m, concatenate per-core aggr blocks.
"""

import math
import sys
from contextlib import ExitStack

import numpy as np

for _p in ("/opt/trn_rl_repo",):
    if _p not in sys.path:
        sys.path.insert(0, _p)

import concourse.bass as bass  # noqa: E402
import concourse.mybir as mybir  # noqa: E402
import concourse.tile as tile  # noqa: E402
from concourse import bacc  # noqa: E402
from concourse.bass_utils import run_bass_kernel_spmd  # noqa: E402

P = 128            # partitions / edges per tile
D = 64             # feature dim
WIN = 128          # nodes per scatter window
MGROUP = 8         # tiles per MLP psum group ([128, 512] = one psum bank)
CHUNK = 32         # tiles per input DMA chunk (1 MiB bf16)

F32 = mybir.dt.float32
BF16 = mybir.dt.bfloat16

# compute dtype for x/W/onehot/m ("f32" or "bf16")
X_DTYPE = BF16
M_DTYPE = BF16


def _np_of(dt):
    return np.dtype(mybir.dt.np(dt))


# ---------------------------------------------------------------------------
# host-side packing
# ---------------------------------------------------------------------------

def _pack(x_i, x_j, recipients, n_nodes, n_cores):
    """Sort/shard/pack edges. Returns (per_core list of dicts, meta dict)."""
    E, d = x_i.shape
    assert d == D
    nodes_per_core = n_nodes // n_cores
    assert nodes_per_core * n_cores == n_nodes
    n_win = math.ceil(nodes_per_core / WIN)

    r = np.asarray(recipients).astype(np.int64).ravel()
    order = np.argsort(r, kind="stable").astype(np.int64)
    r_sorted = r[order]
    core_bounds = np.searchsorted(
        r_sorted, np.arange(n_cores + 1) * nodes_per_core
    )

    # window edge counts per (core, window)
    counts = np.zeros((n_cores, n_win), dtype=np.int64)
    per_core_raw = []
    for c in range(n_cores):
        lo, hi = core_bounds[c], core_bounds[c + 1]
        seg_edges = order[lo:hi]
        ln = r_sorted[lo:hi] - c * nodes_per_core      # local node ids
        win = ln // WIN
        offs = ln - win * WIN
        counts[c] = np.bincount(win, minlength=n_win)
        per_core_raw.append((seg_edges, win, offs))

    # Each core orders its windows by descending edge count; program position
    # g holds every core's g-th largest window, so K_g = max over cores of
    # similarly-ranked counts stays tight (less padding than natural order).
    win_perm = np.argsort(-counts, axis=1, kind="stable")  # [c, pos] -> window
    counts_sorted = -np.sort(-counts, axis=1)
    k_g = np.maximum(np.ceil(counts_sorted / P).astype(np.int64).max(axis=0), 1)
    T = int(k_g.sum())
    T_pad = math.ceil(T / CHUNK) * CHUNK
    n_chunks = T_pad // CHUNK
    n_groups = T_pad // MGROUP

    # tile -> window map (pad tiles attach to the last window)
    tile_window = np.repeat(np.arange(n_win), k_g)
    tile_window = np.concatenate(
        [tile_window, np.full(T_pad - T, n_win - 1, dtype=np.int64)]
    )
    # first/last tile per window (over the padded tile list)
    tile_first = np.zeros(T_pad, dtype=bool)
    tile_last = np.zeros(T_pad, dtype=bool)
    for g in range(n_win):
        idx = np.nonzero(tile_window == g)[0]
        tile_first[idx[0]] = True
        tile_last[idx[-1]] = True

    win_slot0 = np.concatenate([[0], np.cumsum(k_g)]) * P  # slot base per window

    x_np = _np_of(X_DTYPE)
    per_core = []
    for c in range(n_cores):
        seg_edges, win, offs = per_core_raw[c]
        cnt = counts[c]
        rank = np.empty(n_win, dtype=np.int64)  # window -> program position
        rank[win_perm[c]] = np.arange(n_win)
        win_starts = np.concatenate([[0], np.cumsum(cnt)])[:-1]
        pos_in_win = np.arange(len(seg_edges)) - np.repeat(win_starts, cnt)
        slot = win_slot0[rank[win]] + pos_in_win

        slot_edge = np.full(T_pad * P, -1, dtype=np.int64)
        slot_off = np.full(T_pad * P, -1.0, dtype=np.float32)
        slot_edge[slot] = seg_edges
        slot_off[slot] = offs.astype(np.float32)

        xs = np.zeros((T_pad * P, 2 * D), dtype=np.float32)
        valid = slot_edge >= 0
        ve = slot_edge[valid]
        xs[valid, :D] = x_i[ve]
        xs[valid, D:] = x_j[ve]
        # chunk-transposed layout: [n_chunks, 128(k), CHUNK*128(e)]
        xcat = (
            xs.reshape(n_chunks, CHUNK, P, 2 * D)
            .transpose(0, 3, 1, 2)
            .reshape(n_chunks, 2 * D, CHUNK * P)
            .astype(x_np)
        )
        offsT = np.ascontiguousarray(slot_off.reshape(T_pad, P).T).astype(
            x_np
        )  # [128, T_pad]
        per_core.append(
            dict(xcat=xcat, offsT=offsT, slot_edge=slot_edge, win_perm=win_perm[c])
        )

    meta = dict(
        T_pad=T_pad,
        n_chunks=n_chunks,
        n_groups=n_groups,
        n_win=n_win,
        nodes_per_core=nodes_per_core,
        tile_window=tile_window,
        tile_first=tile_first,
        tile_last=tile_last,
    )
    return per_core, meta


# ---------------------------------------------------------------------------
# device program
# ---------------------------------------------------------------------------

def _build_program(meta):
    T_pad = meta["T_pad"]
    n_chunks = meta["n_chunks"]
    n_groups = meta["n_groups"]
    n_win = meta["n_win"]
    tile_window = meta["tile_window"]
    tile_first = meta["tile_first"]
    tile_last = meta["tile_last"]

    nc = bacc.Bacc(None)
    xcat_h = nc.declare_dram_parameter(
        "xcat", [n_chunks, 2 * D, CHUNK * P], X_DTYPE, isOutput=False
    )
    offs_h = nc.declare_dram_parameter("offs", [P, T_pad], X_DTYPE, isOutput=False)
    w_h = nc.declare_dram_parameter("w", [2 * D, D], X_DTYPE, isOutput=False)
    bias_h = nc.declare_dram_parameter(
        "bias", [1, MGROUP * D], X_DTYPE, isOutput=False
    )
    iota_h = nc.declare_dram_parameter(
        "iota", [P, MGROUP * WIN], X_DTYPE, isOutput=False
    )
    assert n_groups % 2 == 0
    m_out_h = nc.declare_dram_parameter(
        "m_out", [n_groups // 2, P, 2 * MGROUP * D], M_DTYPE, isOutput=True
    )
    aggr_h = nc.declare_dram_parameter("aggr", [WIN, n_win * D], F32, isOutput=True)

    with tile.TileContext(nc) as tc, ExitStack() as ctx:
        const_pool = ctx.enter_context(tc.tile_pool(name="const", bufs=1))
        chunk_pool = ctx.enter_context(tc.tile_pool(name="xchunk", bufs=4))
        m_pool = ctx.enter_context(tc.tile_pool(name="m", bufs=4))
        oh_pool = ctx.enter_context(tc.tile_pool(name="onehot", bufs=6))
        mpsum_pool = ctx.enter_context(
            tc.tile_pool(name="mpsum", bufs=4, space="PSUM")
        )
        apsum_pool = ctx.enter_context(
            tc.tile_pool(name="apsum", bufs=3, space="PSUM")
        )

        w_sb = const_pool.tile([2 * D, D], X_DTYPE)
        nc.sync.dma_start(out=w_sb[:], in_=w_h[:, :])
        bias_sb = const_pool.tile([1, MGROUP * D], X_DTYPE)
        nc.sync.dma_start(out=bias_sb[:], in_=bias_h[:, :])
        iota_sb = const_pool.tile([P, MGROUP * WIN], X_DTYPE)
        nc.sync.dma_start(out=iota_sb[:], in_=iota_h[:, :])
        offs_sb = const_pool.tile([P, T_pad], X_DTYPE)
        nc.sync.dma_start(out=offs_sb[:], in_=offs_h[:, :])
        stage_sb = const_pool.tile([WIN, n_win * D], F32)
        ones_sb = const_pool.tile([1, P], X_DTYPE)
        nc.vector.memset(ones_sb[:], 1.0)

        chunk_tiles: dict[int, object] = {}
        m_tiles: dict[int, object] = {}
        oh_tiles: dict[int, object] = {}
        aggr_psum = [None]

        def xslice(t):
            ch = t // CHUNK
            if ch not in chunk_tiles:
                xt = chunk_pool.tile([2 * D, CHUNK * P], X_DTYPE)
                nc.sync.dma_start(out=xt[:], in_=xcat_h[ch])
                chunk_tiles[ch] = xt
            j = t % CHUNK
            return chunk_tiles[ch][:, j * P : (j + 1) * P]

        def emit_mlp(g):
            pm = mpsum_pool.tile([P, MGROUP * D], F32)
            # bias pre-load: ones[1,128].T @ bias[1,512] broadcasts b into psum
            nc.tensor.matmul(
                out=pm[:], lhsT=ones_sb[:], rhs=bias_sb[:], start=True, stop=False
            )
            for j in range(MGROUP):
                t = g * MGROUP + j
                nc.tensor.matmul(
                    out=pm[:, j * D : (j + 1) * D],
                    lhsT=xslice(t),
                    rhs=w_sb[:],
                    start=False,
                    stop=(j == MGROUP - 1),
                )
            # m tiles are paired [128, 1024]: group g occupies half (g%2), one
            # DMA per pair -> 2 KiB descriptors instead of 1 KiB
            if g % 2 == 0:
                mpair = m_pool.tile([P, 2 * MGROUP * D], M_DTYPE)
                m_tiles[g // 2] = mpair
            else:
                mpair = m_tiles[g // 2]
            off = (g % 2) * MGROUP * D
            nc.scalar.activation(
                out=mpair[:, off : off + MGROUP * D],
                in_=pm[:],
                func=mybir.ActivationFunctionType.Relu,
            )
            if g % 2 == 1:
                nc.sync.dma_start(out=m_out_h[g // 2], in_=mpair[:])
            # one-hot build; two halves so the first scatter matmuls can
            # start after half the compare
            oh = oh_pool.tile([P, MGROUP * WIN], X_DTYPE)
            half = MGROUP // 2
            for h in range(2):
                t0 = g * MGROUP + h * half
                nc.vector.tensor_tensor(
                    out=oh[:, h * half * WIN : (h + 1) * half * WIN].rearrange(
                        "p (t w) -> p t w", w=WIN
                    ),
                    in0=iota_sb[:, : half * WIN].rearrange("p (t w) -> p t w", w=WIN),
                    in1=offs_sb[:, t0 : t0 + half].to_broadcast([P, half, WIN]),
                    op=mybir.AluOpType.is_equal,
                )
            oh_tiles[g] = oh

        def emit_scatter(g):
            mpair = m_tiles[g // 2]
            moff = (g % 2) * MGROUP * D
            if g % 2 == 1:
                del m_tiles[g // 2]
            oh = oh_tiles.pop(g)
            for j in range(MGROUP):
                t = g * MGROUP + j
                gw = int(tile_window[t])
                if tile_first[t]:
                    aggr_psum[0] = apsum_pool.tile([WIN, D], F32, name="aggr_psum")
                nc.tensor.matmul(
                    out=aggr_psum[0][:],
                    lhsT=oh[:, j * WIN : (j + 1) * WIN],
                    rhs=mpair[:, moff + j * D : moff + (j + 1) * D],
                    start=bool(tile_first[t]),
                    stop=bool(tile_last[t]),
                )
                if tile_last[t]:
                    nc.scalar.copy(
                        out=stage_sb[:, gw * D : (gw + 1) * D], in_=aggr_psum[0][:]
                    )

        LAG = 3
        for g in range(n_groups):
            emit_mlp(g)
            if g >= LAG:
                emit_scatter(g - LAG)
        for g in range(n_groups - LAG, n_groups):
            emit_scatter(g)

        nc.sync.dma_start(out=aggr_h[:, :], in_=stage_sb[:])

    nc.compile()
    return nc


# ---------------------------------------------------------------------------
# entry point
# ---------------------------------------------------------------------------

def _make_in_maps(per_core, W, b):
    x_np = _np_of(X_DTYPE)
    w_in = np.ascontiguousarray(W.astype(x_np))
    bias_in = np.tile(b[None, :], (1, MGROUP)).astype(x_np)
    iota_in = np.tile(np.arange(WIN, dtype=np.float64), (P, MGROUP)).astype(x_np)
    return [
        {
            "xcat": pc["xcat"],
            "offs": pc["offsT"],
            "w": w_in,
            "bias": bias_in,
            "iota": iota_in,
        }
        for pc in per_core
    ]


def _decode(results, per_core, meta, E, n_nodes):
    n_win = meta["n_win"]
    npc = meta["nodes_per_core"]
    n_groups = meta["n_groups"]
    aggr = np.zeros((n_nodes, D), dtype=np.float32)
    m = np.empty((E, D), dtype=np.float32)
    for c, out in enumerate(results):
        aggr_np = np.asarray(out["aggr"], dtype=np.float32)  # [WIN, n_win*64]
        blocks = aggr_np.reshape(WIN, n_win, D).transpose(1, 0, 2)  # [pos, WIN, 64]
        win_perm = per_core[c]["win_perm"]
        for g in range(n_win):
            w = int(win_perm[g])
            nv = min(WIN, npc - w * WIN)
            aggr[c * npc + w * WIN : c * npc + w * WIN + nv] = blocks[g, :nv]
        m_np = np.asarray(out["m_out"], dtype=np.float32)  # [n_groups/2,128,1024]
        rows = (
            m_np.reshape(n_groups // 2, P, 2 * MGROUP, D)
            .transpose(0, 2, 1, 3)
            .reshape(n_groups * MGROUP * P, D)
        )
        slot_edge = per_core[c]["slot_edge"]
        valid = slot_edge >= 0
        m[slot_edge[valid]] = rows[valid]
    return aggr, m


def _ensure_ntff_hook():
    """Register the axon NTFF profiling hook under ``antenv.axon_hooks``.

    The agent image's ``antenv`` lacks ``axon_hooks`` so trn_boot degrades
    silently; replicate the registration here so trace=True yields
    exec_time_ns. Best-effort — failure just means no trace."""
    import types

    try:
        import antenv.axon_hooks  # noqa: F401
        return
    except ImportError:
        pass
    try:
        import antenv
        from trn_agent_boot.trn_boot import _ntff_profile_via_ctypes

        hook = _ntff_profile_via_ctypes("/opt/axon/libaxon_pjrt.so")
        mod = types.ModuleType("antenv.axon_hooks")
        mod._hook = hook
        mod.get_axon_ntff_profile_hook = lambda: mod._hook
        mod.set_axon_ntff_profile_hook = lambda h: setattr(mod, "_hook", h)
        sys.modules["antenv.axon_hooks"] = mod
        antenv.axon_hooks = mod
    except Exception as e:  # pragma: no cover
        print(f"NTFF hook shim failed ({e}); tracing disabled", file=sys.stderr)


def _run(x_i, x_j, recipients, W, b, n_nodes, n_cores, trace=False):
    if trace:
        _ensure_ntff_hook()
    x_i = np.asarray(x_i, dtype=np.float32)
    x_j = np.asarray(x_j, dtype=np.float32)
    W = np.asarray(W, dtype=np.float32)
    b = np.asarray(b, dtype=np.float32)
    E = x_i.shape[0]

    per_core, meta = _pack(x_i, x_j, recipients, n_nodes, n_cores)
    nc = _build_program(meta)
    in_maps = _make_in_maps(per_core, W, b)

    res = run_bass_kernel_spmd(nc, in_maps, list(range(n_cores)), trace=trace)
    aggr, m = _decode(res.results, per_core, meta, E, n_nodes)
    return (aggr, m), res


def kernel(x_i, x_j, recipients, W, b):
    (aggr, m), _ = _run(x_i, x_j, recipients, W, b, n_nodes=50000, n_cores=8)
    return aggr, m
